# revision 21
# baseline (speedup 1.0000x reference)
"""Causal self-attention (b=2, t=2048, n_embd=768, n_head=12) on 8 TRN2 cores.

Sharding: core c -> batch b = c // 4, head group g = c % 4 (3 heads/group).
Host pre-transposes x (xT, strip-contiguous) and packs all weights in bf16;
each core computes qkv -> per-head causal attention -> partial projection
for its 3 heads; host upcasts + sums the 4 bf16 partial outputs per batch.

Everything on-core runs in bf16 matmuls (1 cycle/row at any free size,
~0.5% rel err, tolerance is 2e-2).  PSUM accumulation stays f32.

Engine assignment:
  PE     warmup, qk, v, scores, att@v, denom broadcast, proj  (~195k cycles)
  ACT    ONLY Exp (+ half the y PSUM->SBUF copies via in-table Copy):
         exactly one ACT_TABLE_LOAD (baseline had 25 at 1.28us each)
  DVE    PSUM->SBUF copies, reciprocal_approx_fast (replaces Ln+Exp),
         normalize multiplies, half the y copies
  Pool   causal trimask multiplies on exp tiles (SBUF-only engine)
  DMA    strip-contiguous loads (6KB descriptors), y stores, h1-shift

The PE clock p-state ramps to 2.4 GHz only after 3us of gapless execution
and resets to 1.2 GHz on any stall, so emission order is driven by a
virtual-time pacer: per attention tile it emits the score matmul, then
enough filler work (qk/v/proj chains for other strips) to cover the exp
latency, then the lagged att@v.  Cross-engine dependencies are modelled so
every PE instruction's inputs are ready well before it issues.  qk/v
chains are emitted atomically (they share the 2 aux PSUM banks; splitting
two same-bank accumulation chains would corrupt PSUM).
"""

import os
import numpy as np
from collections import deque
from contextlib import ExitStack

import ml_dtypes

import concourse.bass as bass
import concourse.mybir as mybir
import concourse.tile as tile
from concourse import bacc
from concourse.bass_utils import run_bass_kernel_spmd

F32 = mybir.dt.float32
F32R = mybir.dt.float32r
BF16 = mybir.dt.bfloat16
AF = mybir.ActivationFunctionType

T = 2048
E = 768
D = 64          # head dim
EC = 6          # e chunks of 128
NS = 4          # q strips of 512
SCALE = 1.0 / 8.0
SEM = 100.0

# virtual-time cost constants (ns) — calibrated from ntff profiles:
# PE at 2.0 GHz (P0) warm = 0.5 ns/row, ~1.2 GHz cold = 0.9 ns/row,
# per-matmul overhead ~3-8 ns; ACT = 1.0 ns/col + 320; DVE cast =
# 1.3 ns/col + 170.
PE_NS = 0.5
COLD_NS = 0.9
WARM_T = 4000.0      # HAM un-throttles ~3.4us after sustained activity
MM_OVH = 8.0


def exp_cost(w):
    return w * 0.833 + 264.0

DVE_QKT = 836.0
DVE_VA = 420.0
DVE_TMP = 830.0
DVE_REC = 800.0
DVE_NORM = 620.0
DVE_Y = 670.0
ACT_Y = 704.0
DVE_MASK = 185.0
POOL_BCAST = 1250.0
DMA_LAT = 2500.0

LAG = 10         # att@v trails its exp by this many attention tiles
NORM_LAG = 8     # norm multiply trails its head's last att@v by this many tiles
BC_LAG = 3       # denom broadcast trails its head's last att@v by this many tiles
PACE_SLACK = 1200.0  # keep the ACT queue within this lead over PE

_CACHED = {}


def build_nc():
    nc = bacc.Bacc("TRN2", target_bir_lowering=False, debug=False)
    xs_d = [
        nc.declare_dram_parameter(f"xs{s}", [128, EC * 512], BF16, isOutput=False)
        for s in range(NS)
    ]
    wqk_d = nc.declare_dram_parameter("wqk", [128, EC * 384], BF16, isOutput=False)
    wv_d = nc.declare_dram_parameter("wv", [128, EC * 192], BF16, isOutput=False)
    wp01_d = nc.declare_dram_parameter("wp01", [128, E], BF16, isOutput=False)
    wp2_d = nc.declare_dram_parameter("wp2", [64, E], BF16, isOutput=False)
    y_d = nc.declare_dram_parameter("y", [T, E], BF16, isOutput=True)

    with tile.TileContext(nc) as tc, ExitStack() as ctx:
        singles = ctx.enter_context(tc.tile_pool(name="singles", bufs=1))
        pool_exp = ctx.enter_context(tc.tile_pool(name="exp", bufs=8))
        pool_tmp = ctx.enter_context(tc.tile_pool(name="tmp", bufs=4))
        pool_rcp = ctx.enter_context(tc.tile_pool(name="rcp", bufs=4))
        pool_den = ctx.enter_context(tc.tile_pool(name="den", bufs=4))
        pool_stg = ctx.enter_context(tc.tile_pool(name="stg", bufs=3))
        pool_bc = ctx.enter_context(tc.tile_pool(name="bcast", bufs=4))
        pool_y = ctx.enter_context(tc.tile_pool(name="yout", bufs=4))
        # PSUM: 2x(2 banks score pairs) + 2 + 2 = 8 banks
        ps_sc = ctx.enter_context(tc.tile_pool(name="pssc", bufs=2, space="PSUM"))
        ps_acc = ctx.enter_context(tc.tile_pool(name="psacc", bufs=2, space="PSUM"))
        ps_aux = ctx.enter_context(tc.tile_pool(name="psaux", bufs=2, space="PSUM"))

        # ---- constants ----
        # scratch (warmup moving operand) first so the PE can start ASAP
        scratch = singles.tile([128, 512], BF16, tag="scratch")
        nc.gpsimd.memset(scratch, 1.0)

        tri_f = singles.tile([128, 128], F32, tag="tri_f")
        nc.gpsimd.memset(tri_f, 1.0)
        nc.gpsimd.affine_select(
            out=tri_f,
            in_=tri_f,
            compare_op=mybir.AluOpType.is_ge,
            fill=0.0,
            base=0,
            pattern=[[1, 128]],
            channel_multiplier=-1,
        )
        trimask = singles.tile([128, 128], BF16, tag="trimask")
        nc.vector.tensor_copy(trimask[:], tri_f[:])

        # ---- weights + x ----
        wqk_sb = singles.tile([128, EC, 384], BF16, tag="wqk_sb")
        wv_sb = singles.tile([128, EC, 192], BF16, tag="wv_sb")
        wp01_sb = singles.tile([128, E], BF16, tag="wp01_sb")
        wp2_sb = singles.tile([64, E], BF16, tag="wp2_sb")
        xs_sb = [
            singles.tile([128, EC, 512], BF16, tag=f"xs_sb{s}", name=f"xs_sb{s}")
            for s in range(NS)
        ]

        nc.sync.dma_start(wqk_sb[:].rearrange("p eo c -> p (eo c)"), wqk_d[:])
        nc.sync.dma_start(xs_sb[0][:].rearrange("p eo t -> p (eo t)"), xs_d[0][:])
        nc.sync.dma_start(wv_sb[:].rearrange("p eo c -> p (eo c)"), wv_d[:])
        for s in range(1, NS):
            nc.sync.dma_start(
                xs_sb[s][:].rearrange("p eo t -> p (eo t)"), xs_d[s][:]
            )
        nc.sync.dma_start(wp01_sb[:], wp01_d[:])
        nc.sync.dma_start(wp2_sb[:], wp2_d[:])

        # ---- persistent intermediates ----
        k01 = singles.tile([128, T], BF16, tag="k01")
        q01 = singles.tile([128, T], BF16, tag="q01")
        kq2 = singles.tile([128, T], BF16, tag="kq2")
        qt2 = singles.tile([64, T], BF16, tag="qt2")
        va = singles.tile([128, 16, 195], BF16, tag="va")
        outT01 = singles.tile([128, T], BF16, tag="outT01")
        outT2 = singles.tile([64, T], BF16, tag="outT2")

        ones16 = singles.tile([128, 16], F32, tag="ones16")
        nc.vector.memset(ones16, 1.0)
        ones_bc = singles.tile([65, 64], F32, tag="ones_bc")
        nc.vector.memset(ones_bc, 1.0)
        for h in range(3):
            nc.vector.tensor_copy(va[:, :, 65 * h + 64], ones16[:])

        # ================= virtual-time pacer =================
        st = {
            "pe": 0.0, "act": 0.0, "dve": 0.0, "pool": 0.0,
            "stall": 0.0, "y_alt": 0, "in_filler": False,
            "act_busy": 0.0, "dve_busy": 0.0, "pool_busy": 0.0, "pe_work": 0.0,
            "pj_slots": [0.0, 0.0], "pj_slot_k": 0,
        }
        stall_sites = {}
        exp_fin = {}    # attention tile index -> exp/mask finish time
        res_ready = {}  # resource name -> ready time

        def pe_busy(c):
            st["pe"] += c
            st["pe_work"] += c

        def mm_cost(w):
            rate = COLD_NS if st["pe"] < WARM_T else PE_NS
            return w * rate + MM_OVH

        def need(t, site=""):
            """PE needs time `t` reached; burn fillers, else stall."""
            if not st["in_filler"]:
                while st["pe"] < t and emit_one_filler():
                    pass
            if st["pe"] < t:
                st["stall"] += t - st["pe"]
                nf = sum(1 for cid, i in fillers if chains[cid][i] is not None)
                key = f"{site}/s{st.get('cur_s', '?')}/f{nf}"
                stall_sites[key] = stall_sites.get(key, 0.0) + (t - st["pe"])
                st["pe"] = t

        def dve(cost, after=None):
            st["dve"] = max(st["dve"], (after or 0.0) + SEM) + cost
            st["dve_busy"] += cost
            return st["dve"]

        def act(cost, after=None):
            st["act"] = max(st["act"], (after or 0.0) + SEM) + cost
            st["act_busy"] += cost
            return st["act"]

        def pool(cost, after=None):
            st["pool"] = max(st["pool"], (after or 0.0) + SEM) + cost
            st["pool_busy"] += cost
            return st["pool"]

        # ---------- filler machinery ----------
        # fillers: deque of chain ids; chains: id -> [unit closures | None].
        # Each unit is atomic (one or more matmuls + trailing engine ops).
        fillers = deque()
        chains = {}

        def add_chain(cid, units):
            chains[cid] = list(units)
            fillers.extend((cid, i) for i in range(len(units)))

        def emit_unit(cid, idx):
            u = chains[cid][idx]
            chains[cid][idx] = None
            st["in_filler"] = True
            u()
            st["in_filler"] = False

        def emit_one_filler():
            for pass_proj in (False, True):
                for k in range(len(fillers)):
                    cid, idx = fillers[k]
                    if cid.startswith("proj") != pass_proj:
                        continue
                    unit = chains[cid][idx]
                    if unit is None:
                        continue
                    ready = getattr(unit, "ready", None)
                    if ready is not None and st["pe"] < ready():
                        continue
                    del fillers[k]
                    emit_unit(cid, idx)
                    return True
            while fillers and chains[fillers[0][0]][fillers[0][1]] is None:
                fillers.popleft()
            return False

        def force_chain(cid):
            ch = chains.get(cid)
            if not ch:
                return
            for i, unit in enumerate(ch):
                if unit is not None:
                    ch[i] = None
                    st["in_filler"] = True
                    unit()
                    st["in_filler"] = False

        # ---------- building blocks ----------
        def qk_chain(s, cc):
            """One atomic unit: 6 qk matmuls for (strip s, chunk cc) + copy."""
            dst = [k01, q01, kq2][cc]

            def f():
                need(res_ready.get(f"xs{s}", 0.0), "xs")
                need(res_ready.get("wqk", 0.0), "wqk")
                pq = ps_aux.tile([128, 512], F32, tag="aux", name=f"qk{s}{cc}")
                for ec in range(EC):
                    nc.tensor.matmul(
                        pq[:],
                        wqk_sb[:, ec, cc * 128 : (cc + 1) * 128],
                        xs_sb[s][:, ec, :],
                        start=(ec == 0),
                        stop=(ec == EC - 1),
                    )
                    pe_busy(mm_cost(512))
                fin = dve(DVE_QKT, after=st["pe"])
                nc.vector.tensor_copy(dst[:, s * 512 : (s + 1) * 512], pq[:])
                res_ready[f"qk{s}{cc}"] = fin
                if cc == 2:
                    nc.sync.dma_start(
                        qt2[0:64, s * 512 : (s + 1) * 512],
                        kq2[64:128, s * 512 : (s + 1) * 512],
                    )
                    res_ready[f"qt2{s}"] = fin + DMA_LAT

            f.ready = lambda: max(
                res_ready.get(f"xs{s}", 0.0), res_ready.get("wqk", 0.0)
            )
            return [f]

        def v_chain(s, j):
            """One atomic unit: 6 v matmuls for t-chunk 4s+j + va copy."""
            t_i = 4 * s + j

            def f():
                need(res_ready.get(f"xs{s}", 0.0), "xs")
                need(res_ready.get("wv", 0.0), "wv")
                pv = ps_aux.tile([128, 512], F32, tag="aux", name=f"v{t_i}")
                for ec in range(EC):
                    nc.tensor.matmul(
                        pv[:, 0:192],
                        xs_sb[s][:, ec, j * 128 : (j + 1) * 128],
                        wv_sb[:, ec, :],
                        start=(ec == 0),
                        stop=(ec == EC - 1),
                    )
                    pe_busy(mm_cost(192))
                fin = dve(DVE_VA, after=st["pe"])
                nc.vector.tensor_copy(
                    va[:, t_i, :].rearrange("p (h c) -> p h c", c=65)[:, :, 0:64],
                    pv[:, 0:192].rearrange("p (h c) -> p h c", c=64),
                )
                res_ready[f"va{t_i}"] = fin

            f.ready = lambda: max(
                res_ready.get(f"xs{s}", 0.0), res_ready.get("wv", 0.0)
            )
            return [f]

        def queue_proj(s):
            units = []
            for j in range(4):
                t_i = 4 * s + j
                y_sb = pool_y.tile([128, E], BF16, tag="y", name=f"y{s}{j}")

                def unit(t_i=t_i, eh=None, y_sb=y_sb):
                    def f(eh=eh):
                        need(res_ready.get(f"outT{s}", 0.0), "outT")
                        need(res_ready.get("wp", 0.0), "wp")
                        slots = st["pj_slots"]
                        k = min(range(len(slots)), key=lambda i: slots[i])
                        need(slots[k], "pjbank")
                        pp = ps_aux.tile(
                            [128, 512], F32, tag="aux", name=f"pp{t_i}{eh}",
                        )
                        st["pj_slot_k"] = k
                        nc.tensor.matmul(
                            pp[:, 0:384],
                            outT01[:, t_i * 128 : (t_i + 1) * 128],
                            wp01_sb[:, eh * 384 : (eh + 1) * 384],
                            start=True,
                            stop=False,
                        )
                        pe_busy(mm_cost(384))
                        nc.tensor.matmul(
                            pp[:, 0:384],
                            outT2[0:64, t_i * 128 : (t_i + 1) * 128],
                            wp2_sb[0:64, eh * 384 : (eh + 1) * 384],
                            start=False,
                            stop=True,
                        )
                        pe_busy(mm_cost(384))
                        if st["pe"] - st["act"] > 1200.0:
                            yfin = act(ACT_Y, after=st["pe"])
                            nc.scalar.copy(
                                y_sb[:, eh * 384 : (eh + 1) * 384], pp[:, 0:384]
                            )
                        else:
                            yfin = dve(DVE_Y, after=st["pe"])
                            nc.vector.tensor_copy(
                                y_sb[:, eh * 384 : (eh + 1) * 384], pp[:, 0:384]
                            )
                        st["pj_slots"][st["pj_slot_k"]] = yfin + SEM
                        if eh == 1:
                            nc.sync.dma_start(
                                y_d[t_i * 128 : (t_i + 1) * 128, :], y_sb[:]
                            )

                    f.ready = lambda: max(
                        res_ready.get(f"outT{s}", 0.0),
                        res_ready.get("wp", 0.0),
                        min(st["pj_slots"]),
                    )
                    return f

                units.append(unit(eh=0))
                units.append(unit(eh=1))
            add_chain(f"proj{s}", units)

        # ---------- attention ----------
        def head_aps(h, kc, s, o):
            if h == 0:
                return (
                    k01[0:64, kc * 128 : (kc + 1) * 128],
                    q01[0:64, s * 512 + o : (s + 1) * 512],
                )
            if h == 1:
                return (
                    k01[64:128, kc * 128 : (kc + 1) * 128],
                    q01[64:128, s * 512 + o : (s + 1) * 512],
                )
            return (
                kq2[0:64, kc * 128 : (kc + 1) * 128],
                qt2[0:64, s * 512 + o : (s + 1) * 512],
            )

        gidx = 0
        pending_av = deque()

        def emit_av():
            i, h, kc, n, acc, expT, idx, o, w = pending_av.popleft()
            need(exp_fin[i] + SEM + 200.0, "av_exp")
            need(res_ready.get(f"va{kc}", 0.0) + SEM, "av_va")
            nc.tensor.matmul(
                acc[0:65, o:512],
                va[:, kc, h * 65 : h * 65 + 65],
                expT[:, idx, o:512],
                start=(kc == 0),
                stop=(kc == n - 1),
            )
            pe_busy(mm_cost(w))

        def finish_head(s, h, acc):
            while pending_av:
                emit_av()
            av_done = st["pe"]
            tmp = pool_tmp.tile([128, 512], F32, tag="tmp", name=f"tmp{s}{h}")
            tfin = dve(DVE_TMP, after=av_done)
            nc.vector.tensor_copy(tmp[0:65, :], acc[0:65, :])

            def apply_norm(nfin, bc_ap):
                if h == 0:
                    nc.vector.tensor_mul(
                        outT01[0:64, s * 512 : (s + 1) * 512],
                        tmp[0:64, :],
                        bc_ap,
                    )
                elif h == 2:
                    nc.vector.tensor_mul(
                        outT2[0:64, s * 512 : (s + 1) * 512],
                        tmp[0:64, :],
                        bc_ap,
                    )
                else:
                    stg = pool_stg.tile([64, 512], BF16, tag="stg", name=f"stg{s}")
                    nc.vector.tensor_mul(stg[:], tmp[0:64, :], bc_ap)
                    nc.sync.dma_start(
                        outT01[64:128, s * 512 : (s + 1) * 512], stg[:]
                    )
                    nfin += DMA_LAT
                res_ready[f"outT{s}"] = max(res_ready.get(f"outT{s}", 0.0), nfin)
                if h == 2:
                    queue_proj(s)

            if s == NS - 1:
                # tail fast path: PE broadcasts the denominator row from
                # partition 64 down to partitions 0-63 (contraction-1 f32
                # matmul with a ones column), then DVE reciprocal on the
                # broadcast -- skips the partition-hop DMA latency on the
                # critical tail.
                rcp_t = pool_rcp.tile([64, 512], F32, tag="rcp", name=f"rcpt{h}")
                rfin_est = tfin + 900.0

                def norm_unit():
                    need(tfin + SEM, "bcmm")
                    bcp = ps_aux.tile([64, 512], F32, tag="aux", name=f"bcp{h}")
                    nc.tensor.matmul(
                        bcp[:], ones_bc[64:65, 0:64], tmp[64:65, :],
                        start=True, stop=True,
                    )
                    pe_busy(512.0 + MM_OVH + 256.0)  # f32 moving: half rate
                    rfin = dve(DVE_REC, after=st["pe"] + SEM)
                    nc.vector.reciprocal_approx_fast(rcp_t[:], bcp[:])
                    nfin = dve(DVE_NORM, after=rfin)
                    apply_norm(nfin, rcp_t[:])

                return norm_unit, rfin_est

            # steady path: DVE recip and gpsimd partition_broadcast only
            # honor partition 0, so hop the denominator row down via DMA.
            den = pool_den.tile([1, 512], F32, tag="den", name=f"den{s}{h}")
            nc.sync.dma_start(den[:], tmp[64:65, :])
            dfin = tfin + DMA_LAT
            rcp = pool_rcp.tile([1, 512], F32, tag="rcp", name=f"rcp{s}{h}")
            rfin_est = dfin + DVE_REC + 500.0

            def norm_unit():
                rfin = dve(DVE_REC, after=dfin)
                nc.vector.reciprocal_approx_fast(rcp[:], den[:])
                bcast = pool_bc.tile(
                    [64, 512], F32, tag="bcast", name=f"bc{s}{h}"
                )
                bcfin = pool(POOL_BCAST, after=rfin)
                nc.gpsimd.partition_broadcast(bcast[:], rcp[:])
                nfin = dve(DVE_NORM, after=bcfin)
                apply_norm(nfin, bcast[:])

            return norm_unit, rfin_est

        # ================= emission =================
        # measured DMA landing times relative to first warm matmul (~7us
        # real): wqk ~+2.3us, xs strips stream in 10.5-20us real, wp last
        res_ready["wqk"] = 2300.0
        res_ready["wv"] = 6000.0
        res_ready["wp"] = 12800.0
        for s in range(NS):
            res_ready[f"xs{s}"] = 4600.0 + s * 2400.0

        # warmup chain: hold PE activity (HAM) while the first DMAs land.
        # 32-col stationary -> 1/4 array energy, same occupancy.
        warm = ps_aux.tile([32, 512], F32, tag="aux", name="warm")
        NWARM = 12
        for i in range(NWARM):
            nc.tensor.matmul(
                warm[:],
                scratch[:, 0:32],
                scratch[:],
                start=(i == 0),
                stop=(i == NWARM - 1),
            )
            pe_busy(512 * (1.54 if i == 0 else COLD_NS) + MM_OVH)

        # strip-0 prep up front
        add_chain("qk01", qk_chain(0, 1))
        add_chain("qk00", qk_chain(0, 0))
        add_chain("qk02", qk_chain(0, 2))
        for j in range(4):
            add_chain(f"v0{j}", v_chain(0, j))
        for cid in ("qk01", "qk00", "qk02", "v00", "v01", "v02", "v03"):
            force_chain(cid)

        # remaining strips' prep chains: inventory for the pacer, consumed
        # as filler or force-emitted at their use deadlines
        for s2 in range(1, NS):
            add_chain(f"qk{s2}1", qk_chain(s2, 1))
            add_chain(f"qk{s2}0", qk_chain(s2, 0))
            add_chain(f"qk{s2}2", qk_chain(s2, 2))
            for j in range(4):
                add_chain(f"v{s2}{j}", v_chain(s2, j))

        sc_hist = deque(maxlen=3)  # exp-read times of the 3 score banks
        deferred = deque()         # (due_gidx, fn) engine items woven in later

        HEAD_ORDER = [(s, h) for s in range(NS) for h in range(3)]
        for s, h in HEAD_ORDER:
            st["cur_s"] = s
            n = 4 * (s + 1)
            if h == 0:
                force_chain(f"qk{s}1")
                need(res_ready.get(f"qk{s}1", 0.0), "q01")
            if h == 2:
                force_chain(f"qk{s}2")
                need(res_ready.get(f"qk{s}2", 0.0), "qk2")
                need(res_ready.get(f"qt2{s}", 0.0), "qt2")

            acc = ps_acc.tile([128, 512], F32, tag="acc", name=f"acc{s}{h}")
            for kp in range(n // 2):
                pair = (2 * kp, 2 * kp + 1)
                js = [kc - 4 * s for kc in pair]
                os_ = [0 if j < 0 else j * 128 for j in js]
                ws = [512 - o for o in os_]
                for kc, j in zip(pair, js):
                    if j >= 0:
                        force_chain(f"qk{s}0")
                        force_chain(f"v{s}{j}")
                        if h == 2:
                            force_chain(f"qk{s}2")
                        need(
                            res_ready.get(
                                f"qk{s}0" if h < 2 else f"qk{s}2", 0.0
                            ),
                            "kdiag",
                        )
                if len(sc_hist) == 2:
                    need(sc_hist[0] + SEM, "scbank")
                pss = ps_sc.tile([128, 2, 512], F32, tag="sc", name=f"ps{gidx}")
                expT = pool_exp.tile(
                    [128, 2, 512], BF16, tag="expT", name=f"e{gidx}"
                )
                for idx in range(2):
                    lhs, rhs = head_aps(h, pair[idx], s, os_[idx])
                    nc.tensor.matmul(
                        pss[:, idx, os_[idx] : 512], lhs, rhs,
                        start=True, stop=True,
                    )
                    pe_busy(mm_cost(ws[idx]))
                if os_[0] == os_[1]:
                    # one ACT op exps both banks of the pair tile
                    efin = act(exp_cost(2 * ws[0]), after=st["pe"])
                    nc.scalar.activation(
                        expT[:, :, os_[0] : 512], pss[:, :, os_[0] : 512],
                        AF.Exp, scale=SCALE,
                    )
                    efins = [efin, efin]
                else:
                    efins = []
                    for idx in range(2):
                        efins.append(act(exp_cost(ws[idx]), after=st["pe"]))
                        nc.scalar.activation(
                            expT[:, idx, os_[idx] : 512],
                            pss[:, idx, os_[idx] : 512],
                            AF.Exp, scale=SCALE,
                        )
                sc_hist.append(max(efins))  # pair bank frees when exp read it
                for idx in range(2):
                    kc, j, o, w = pair[idx], js[idx], os_[idx], ws[idx]
                    efin = efins[idx]
                    if j >= 0:
                        # DVE, not GpSimd: the GpSimd sequencer burns
                        # 0.3-1.9us per semaphore wait + library reloads
                        efin = dve(DVE_MASK, after=efin)
                        nc.vector.tensor_mul(
                            expT[:, idx, o : o + 128],
                            expT[:, idx, o : o + 128],
                            trimask[:],
                        )
                    exp_fin[gidx] = efin
                    pending_av.append((gidx, h, kc, n, acc, expT, idx, o, w))
                    gidx += 1
                while (
                    deferred
                    and deferred[0][0] <= gidx
                    and deferred[0][1] <= st["pe"]
                ):
                    deferred.popleft()[2]()
                while len(pending_av) > LAG:
                    emit_av()
                while st["act"] > st["pe"] + PACE_SLACK and emit_one_filler():
                    pass

            norm_unit, rfin_est = finish_head(s, h, acc)
            deferred.append((gidx + NORM_LAG, rfin_est, norm_unit))

        # keep the PE clock hot through the tail normalization chain
        # (32-col stationary: 1/4 array energy)
        tail_warm = ps_sc.tile([32, 512], F32, tag="sc", name="tail_warm")
        NTAIL = 16
        for i in range(NTAIL):
            nc.tensor.matmul(
                tail_warm[:],
                scratch[:, 0:32],
                scratch[:],
                start=(i == 0),
                stop=(i == NTAIL - 1),
            )
            pe_busy(mm_cost(512))
        while deferred:
            deferred.popleft()[2]()
        while emit_one_filler():
            pass
        for cid in list(chains):
            force_chain(cid)

        print(
            f"[pacer] pe={st['pe']/1e3:.1f}us (work {st['pe_work']/1e3:.1f}) "
            f"act={st['act']/1e3:.1f}us (busy {st['act_busy']/1e3:.1f}) "
            f"dve busy {st['dve_busy']/1e3:.1f} pool busy {st['pool_busy']/1e3:.1f} "
            f"stall={st['stall']/1e3:.2f}us"
        )
        print("[pacer] stalls:", {k: round(v/1e3, 2) for k, v in sorted(stall_sites.items(), key=lambda kv: -kv[1])})

    nc.compile()
    return nc


def _shard_inputs(x, w_qkv, w_proj):
    bf16 = ml_dtypes.bfloat16
    in_maps = []
    for c in range(8):
        b, g = c // 4, c % 4
        h0 = 3 * g

        def strip_pack(arr2d, cols):
            # [768, cols] -> [128, 6*cols]: row p = concat over eo of
            # arr2d[eo*128 + p, :]
            a = (
                arr2d.reshape(EC, 128, cols)
                .transpose(1, 0, 2)
                .reshape(128, EC * cols)
            )
            return np.ascontiguousarray(a.astype(bf16))

        xT = x[b].T  # [768, 2048]
        m = {}
        for s in range(NS):
            m[f"xs{s}"] = strip_pack(xT[:, s * 512 : (s + 1) * 512], 512)

        q = slice(h0 * D, (h0 + 2) * D)
        k = slice(E + h0 * D, E + (h0 + 2) * D)
        wqk = np.concatenate(
            [
                w_qkv[:, k],                                    # k_h0 | k_h1
                w_qkv[:, q],                                    # q_h0 | q_h1
                w_qkv[:, E + (h0 + 2) * D : E + (h0 + 3) * D],  # k_h2
                w_qkv[:, (h0 + 2) * D : (h0 + 3) * D],          # q_h2
            ],
            axis=1,
        )
        m["wqk"] = strip_pack(wqk, 384)
        wv = w_qkv[:, 2 * E + h0 * D : 2 * E + (h0 + 3) * D]    # v_h0|v_h1|v_h2
        m["wv"] = strip_pack(wv, 192)
        m["wp01"] = np.ascontiguousarray(
            w_proj[h0 * D : (h0 + 2) * D, :].astype(bf16)
        )
        m["wp2"] = np.ascontiguousarray(
            w_proj[(h0 + 2) * D : (h0 + 3) * D, :].astype(bf16)
        )
        in_maps.append(m)
    return in_maps


def kernel(x, w_qkv, w_proj):
    x = np.asarray(x, dtype=np.float32)
    w_qkv = np.asarray(w_qkv, dtype=np.float32)
    w_proj = np.asarray(w_proj, dtype=np.float32)

    if "nc" not in _CACHED:
        _CACHED["nc"] = build_nc()
    nc = _CACHED["nc"]

    in_maps = _shard_inputs(x, w_qkv, w_proj)
    trace = bool(int(os.environ.get("KERNEL_TRACE", "0")))
    res = run_bass_kernel_spmd(nc, in_maps, core_ids=list(range(8)), trace=trace)
    _CACHED["last_results"] = res

    y = np.zeros((2, T, E), dtype=np.float32)
    for c in range(8):
        y[c // 4] += np.asarray(res.results[c]["y"], dtype=np.float32)
    return y



# revision 23
# speedup vs baseline: 1.0804x; 1.0804x over previous
"""Causal self-attention (b=2, t=2048, n_embd=768, n_head=12) on 8 TRN2 cores.

Sharding: core c -> batch b = c // 4, head group g = c % 4 (3 heads/group).
Host pre-transposes x (xT, strip-contiguous) and packs all weights in bf16;
each core computes qkv -> per-head causal attention -> partial projection
for its 3 heads; host upcasts + sums the 4 bf16 partial outputs per batch.

Everything on-core runs in bf16 matmuls (1 cycle/row at any free size,
~0.5% rel err, tolerance is 2e-2).  PSUM accumulation stays f32.

Engine assignment:
  PE     warmup, qk, v, scores, att@v, denom broadcast, proj  (~195k cycles)
  ACT    ONLY Exp (+ half the y PSUM->SBUF copies via in-table Copy):
         exactly one ACT_TABLE_LOAD (baseline had 25 at 1.28us each)
  DVE    PSUM->SBUF copies, reciprocal_approx_fast (replaces Ln+Exp),
         normalize multiplies, half the y copies
  Pool   causal trimask multiplies on exp tiles (SBUF-only engine)
  DMA    strip-contiguous loads (6KB descriptors), y stores, h1-shift

The PE clock p-state ramps to 2.4 GHz only after 3us of gapless execution
and resets to 1.2 GHz on any stall, so emission order is driven by a
virtual-time pacer: per attention tile it emits the score matmul, then
enough filler work (qk/v/proj chains for other strips) to cover the exp
latency, then the lagged att@v.  Cross-engine dependencies are modelled so
every PE instruction's inputs are ready well before it issues.  qk/v
chains are emitted atomically (they share the 2 aux PSUM banks; splitting
two same-bank accumulation chains would corrupt PSUM).
"""

import os
import numpy as np
from collections import deque
from contextlib import ExitStack

import ml_dtypes

import concourse.bass as bass
import concourse.mybir as mybir
import concourse.tile as tile
from concourse import bacc
from concourse.bass_utils import run_bass_kernel_spmd

F32 = mybir.dt.float32
F32R = mybir.dt.float32r
BF16 = mybir.dt.bfloat16
AF = mybir.ActivationFunctionType

T = 2048
E = 768
D = 64          # head dim
EC = 6          # e chunks of 128
NS = 4          # q strips of 512
SCALE = 1.0 / 8.0
SEM = 100.0

# virtual-time cost constants (ns) — calibrated from ntff profiles:
# PE at 2.0 GHz (P0) warm = 0.5 ns/row, ~1.2 GHz cold = 0.9 ns/row,
# per-matmul overhead ~3-8 ns; ACT = 1.0 ns/col + 320; DVE cast =
# 1.3 ns/col + 170.
PE_NS = 0.5
COLD_NS = 0.9
WARM_T = 4000.0      # HAM un-throttles ~3.4us after sustained activity
MM_OVH = 8.0


def exp_cost(w):
    return w * 0.833 + 264.0

DVE_QKT = 836.0
DVE_VA = 420.0
DVE_TMP = 830.0
DVE_REC = 800.0
DVE_NORM = 620.0
DVE_Y = 670.0
ACT_Y = 704.0
DVE_MASK = 185.0
POOL_BCAST = 1250.0
DMA_LAT = 2500.0

LAG = 8          # att@v trails its exp by this many attention tiles
NORM_LAG = 8     # norm multiply trails its head's last att@v by this many tiles
BC_LAG = 3       # denom broadcast trails its head's last att@v by this many tiles
PACE_SLACK = 900.0   # keep the ACT queue within this lead over PE

_CACHED = {}


def build_nc():
    nc = bacc.Bacc("TRN2", target_bir_lowering=False, debug=False)
    xs_d = [
        nc.declare_dram_parameter(f"xs{s}", [128, EC * 512], BF16, isOutput=False)
        for s in range(NS)
    ]
    wqk_d = nc.declare_dram_parameter("wqk", [128, EC * 384], BF16, isOutput=False)
    wv_d = nc.declare_dram_parameter("wv", [128, EC * 192], BF16, isOutput=False)
    wp01_d = nc.declare_dram_parameter("wp01", [128, E], BF16, isOutput=False)
    wp2_d = nc.declare_dram_parameter("wp2", [64, E], BF16, isOutput=False)
    y_d = nc.declare_dram_parameter("y", [T, E], BF16, isOutput=True)

    with tile.TileContext(nc) as tc, ExitStack() as ctx:
        singles = ctx.enter_context(tc.tile_pool(name="singles", bufs=1))
        pool_exp = ctx.enter_context(tc.tile_pool(name="exp", bufs=8))
        pool_tmp = ctx.enter_context(tc.tile_pool(name="tmp", bufs=4))
        pool_rcp = ctx.enter_context(tc.tile_pool(name="rcp", bufs=4))
        pool_den = ctx.enter_context(tc.tile_pool(name="den", bufs=4))
        pool_stg = ctx.enter_context(tc.tile_pool(name="stg", bufs=3))
        pool_bc = ctx.enter_context(tc.tile_pool(name="bcast", bufs=4))
        pool_y = ctx.enter_context(tc.tile_pool(name="yout", bufs=4))
        # PSUM: 2x(2 banks score pairs) + 2 + 2 = 8 banks
        ps_sc = ctx.enter_context(tc.tile_pool(name="pssc", bufs=2, space="PSUM"))
        ps_acc = ctx.enter_context(tc.tile_pool(name="psacc", bufs=2, space="PSUM"))
        ps_aux = ctx.enter_context(tc.tile_pool(name="psaux", bufs=2, space="PSUM"))

        # ---- constants ----
        # scratch (warmup moving operand) first so the PE can start ASAP
        scratch = singles.tile([128, 512], BF16, tag="scratch")
        nc.gpsimd.memset(scratch, 1.0)

        tri_f = singles.tile([128, 128], F32, tag="tri_f")
        nc.gpsimd.memset(tri_f, 1.0)
        nc.gpsimd.affine_select(
            out=tri_f,
            in_=tri_f,
            compare_op=mybir.AluOpType.is_ge,
            fill=0.0,
            base=0,
            pattern=[[1, 128]],
            channel_multiplier=-1,
        )
        trimask = singles.tile([128, 128], BF16, tag="trimask")
        nc.vector.tensor_copy(trimask[:], tri_f[:])

        # ---- weights + x ----
        wqk_sb = singles.tile([128, EC, 384], BF16, tag="wqk_sb")
        wv_sb = singles.tile([128, EC, 192], BF16, tag="wv_sb")
        wp01_sb = singles.tile([128, E], BF16, tag="wp01_sb")
        wp2_sb = singles.tile([64, E], BF16, tag="wp2_sb")
        xs_sb = [
            singles.tile([128, EC, 512], BF16, tag=f"xs_sb{s}", name=f"xs_sb{s}")
            for s in range(NS)
        ]

        nc.sync.dma_start(wqk_sb[:].rearrange("p eo c -> p (eo c)"), wqk_d[:])
        nc.sync.dma_start(xs_sb[0][:].rearrange("p eo t -> p (eo t)"), xs_d[0][:])
        nc.sync.dma_start(wv_sb[:].rearrange("p eo c -> p (eo c)"), wv_d[:])
        for s in range(1, NS):
            nc.sync.dma_start(
                xs_sb[s][:].rearrange("p eo t -> p (eo t)"), xs_d[s][:]
            )
        nc.sync.dma_start(wp01_sb[:], wp01_d[:])
        nc.sync.dma_start(wp2_sb[:], wp2_d[:])

        # ---- persistent intermediates ----
        k01 = singles.tile([128, T], BF16, tag="k01")
        q01 = singles.tile([128, T], BF16, tag="q01")
        kq2 = singles.tile([128, T], BF16, tag="kq2")
        qt2 = singles.tile([64, T], BF16, tag="qt2")
        va = singles.tile([128, 16, 195], BF16, tag="va")
        outT01 = singles.tile([128, T], BF16, tag="outT01")
        outT2 = singles.tile([64, T], BF16, tag="outT2")

        ones16 = singles.tile([128, 16], F32, tag="ones16")
        nc.vector.memset(ones16, 1.0)
        ones_bc = singles.tile([65, 64], F32, tag="ones_bc")
        nc.vector.memset(ones_bc, 1.0)
        for h in range(3):
            nc.vector.tensor_copy(va[:, :, 65 * h + 64], ones16[:])

        # ================= virtual-time pacer =================
        st = {
            "pe": 0.0, "act": 0.0, "dve": 0.0, "pool": 0.0,
            "stall": 0.0, "y_alt": 0, "in_filler": False,
            "act_busy": 0.0, "dve_busy": 0.0, "pool_busy": 0.0, "pe_work": 0.0,
            "pj_slots": [0.0, 0.0], "pj_slot_k": 0,
        }
        stall_sites = {}
        exp_fin = {}    # attention tile index -> exp/mask finish time
        res_ready = {}  # resource name -> ready time

        def pe_busy(c):
            st["pe"] += c
            st["pe_work"] += c

        def mm_cost(w):
            rate = COLD_NS if st["pe"] < WARM_T else PE_NS
            return w * rate + MM_OVH

        def need(t, site=""):
            """PE needs time `t` reached; burn fillers, else stall."""
            if not st["in_filler"]:
                while st["pe"] < t and emit_one_filler():
                    pass
            if st["pe"] < t:
                st["stall"] += t - st["pe"]
                nf = sum(1 for cid, i in fillers if chains[cid][i] is not None)
                key = f"{site}/s{st.get('cur_s', '?')}/f{nf}"
                stall_sites[key] = stall_sites.get(key, 0.0) + (t - st["pe"])
                st["pe"] = t

        def dve(cost, after=None):
            st["dve"] = max(st["dve"], (after or 0.0) + SEM) + cost
            st["dve_busy"] += cost
            return st["dve"]

        def act(cost, after=None):
            st["act"] = max(st["act"], (after or 0.0) + SEM) + cost
            st["act_busy"] += cost
            return st["act"]

        def pool(cost, after=None):
            st["pool"] = max(st["pool"], (after or 0.0) + SEM) + cost
            st["pool_busy"] += cost
            return st["pool"]

        # ---------- filler machinery ----------
        # fillers: deque of chain ids; chains: id -> [unit closures | None].
        # Each unit is atomic (one or more matmuls + trailing engine ops).
        fillers = deque()
        chains = {}

        def add_chain(cid, units):
            chains[cid] = list(units)
            fillers.extend((cid, i) for i in range(len(units)))

        def emit_unit(cid, idx):
            u = chains[cid][idx]
            chains[cid][idx] = None
            st["in_filler"] = True
            u()
            st["in_filler"] = False

        def emit_one_filler():
            for pass_proj in (False, True):
                for k in range(len(fillers)):
                    cid, idx = fillers[k]
                    if cid.startswith("proj") != pass_proj:
                        continue
                    unit = chains[cid][idx]
                    if unit is None:
                        continue
                    ready = getattr(unit, "ready", None)
                    if ready is not None and st["pe"] < ready():
                        continue
                    del fillers[k]
                    emit_unit(cid, idx)
                    return True
            while fillers and chains[fillers[0][0]][fillers[0][1]] is None:
                fillers.popleft()
            return False

        def force_chain(cid):
            ch = chains.get(cid)
            if not ch:
                return
            for i, unit in enumerate(ch):
                if unit is not None:
                    ch[i] = None
                    st["in_filler"] = True
                    unit()
                    st["in_filler"] = False

        # ---------- building blocks ----------
        def qk_chain(s, cc):
            """One atomic unit: 6 qk matmuls for (strip s, chunk cc) + copy."""
            dst = [k01, q01, kq2][cc]

            def f():
                need(res_ready.get(f"xs{s}", 0.0), "xs")
                need(res_ready.get("wqk", 0.0), "wqk")
                pq = ps_aux.tile([128, 512], F32, tag="aux", name=f"qk{s}{cc}")
                for ec in range(EC):
                    nc.tensor.matmul(
                        pq[:],
                        wqk_sb[:, ec, cc * 128 : (cc + 1) * 128],
                        xs_sb[s][:, ec, :],
                        start=(ec == 0),
                        stop=(ec == EC - 1),
                    )
                    pe_busy(mm_cost(512))
                fin = dve(DVE_QKT, after=st["pe"])
                nc.vector.tensor_copy(dst[:, s * 512 : (s + 1) * 512], pq[:])
                res_ready[f"qk{s}{cc}"] = fin
                if cc == 2:
                    nc.sync.dma_start(
                        qt2[0:64, s * 512 : (s + 1) * 512],
                        kq2[64:128, s * 512 : (s + 1) * 512],
                    )
                    res_ready[f"qt2{s}"] = fin + DMA_LAT

            f.ready = lambda: max(
                res_ready.get(f"xs{s}", 0.0), res_ready.get("wqk", 0.0)
            )
            return [f]

        def v_chain(s, j):
            """One atomic unit: 6 v matmuls for t-chunk 4s+j + va copy."""
            t_i = 4 * s + j

            def f():
                need(res_ready.get(f"xs{s}", 0.0), "xs")
                need(res_ready.get("wv", 0.0), "wv")
                pv = ps_aux.tile([128, 512], F32, tag="aux", name=f"v{t_i}")
                for ec in range(EC):
                    nc.tensor.matmul(
                        pv[:, 0:192],
                        xs_sb[s][:, ec, j * 128 : (j + 1) * 128],
                        wv_sb[:, ec, :],
                        start=(ec == 0),
                        stop=(ec == EC - 1),
                    )
                    pe_busy(mm_cost(192))
                fin = dve(DVE_VA, after=st["pe"])
                nc.vector.tensor_copy(
                    va[:, t_i, :].rearrange("p (h c) -> p h c", c=65)[:, :, 0:64],
                    pv[:, 0:192].rearrange("p (h c) -> p h c", c=64),
                )
                res_ready[f"va{t_i}"] = fin

            f.ready = lambda: max(
                res_ready.get(f"xs{s}", 0.0), res_ready.get("wv", 0.0)
            )
            return [f]

        def queue_proj(s):
            units = []
            for j in range(4):
                t_i = 4 * s + j
                y_sb = pool_y.tile([128, E], BF16, tag="y", name=f"y{s}{j}")

                def unit(t_i=t_i, eh=None, y_sb=y_sb):
                    def f(eh=eh):
                        need(res_ready.get(f"outT{s}", 0.0), "outT")
                        need(res_ready.get("wp", 0.0), "wp")
                        slots = st["pj_slots"]
                        k = min(range(len(slots)), key=lambda i: slots[i])
                        need(slots[k], "pjbank")
                        pp = ps_aux.tile(
                            [128, 512], F32, tag="aux", name=f"pp{t_i}{eh}",
                        )
                        st["pj_slot_k"] = k
                        nc.tensor.matmul(
                            pp[:, 0:384],
                            outT01[:, t_i * 128 : (t_i + 1) * 128],
                            wp01_sb[:, eh * 384 : (eh + 1) * 384],
                            start=True,
                            stop=False,
                        )
                        pe_busy(mm_cost(384))
                        nc.tensor.matmul(
                            pp[:, 0:384],
                            outT2[0:64, t_i * 128 : (t_i + 1) * 128],
                            wp2_sb[0:64, eh * 384 : (eh + 1) * 384],
                            start=False,
                            stop=True,
                        )
                        pe_busy(mm_cost(384))
                        if st["pe"] - st["act"] > 1200.0:
                            yfin = act(ACT_Y, after=st["pe"])
                            nc.scalar.copy(
                                y_sb[:, eh * 384 : (eh + 1) * 384], pp[:, 0:384]
                            )
                        else:
                            yfin = dve(DVE_Y, after=st["pe"])
                            nc.vector.tensor_copy(
                                y_sb[:, eh * 384 : (eh + 1) * 384], pp[:, 0:384]
                            )
                        st["pj_slots"][st["pj_slot_k"]] = yfin + SEM
                        if eh == 1:
                            nc.sync.dma_start(
                                y_d[t_i * 128 : (t_i + 1) * 128, :], y_sb[:]
                            )

                    f.ready = lambda: max(
                        res_ready.get(f"outT{s}", 0.0),
                        res_ready.get("wp", 0.0),
                        min(st["pj_slots"]),
                    )
                    return f

                units.append(unit(eh=0))
                units.append(unit(eh=1))
            add_chain(f"proj{s}", units)

        # ---------- attention ----------
        def head_aps(h, kc, s, o):
            if h == 0:
                return (
                    k01[0:64, kc * 128 : (kc + 1) * 128],
                    q01[0:64, s * 512 + o : (s + 1) * 512],
                )
            if h == 1:
                return (
                    k01[64:128, kc * 128 : (kc + 1) * 128],
                    q01[64:128, s * 512 + o : (s + 1) * 512],
                )
            return (
                kq2[0:64, kc * 128 : (kc + 1) * 128],
                qt2[0:64, s * 512 + o : (s + 1) * 512],
            )

        gidx = 0
        pending_av = deque()

        def emit_av():
            i, h, kc, n, acc, expT, idx, o, w = pending_av.popleft()
            need(exp_fin[i] + SEM + 200.0, "av_exp")
            need(res_ready.get(f"va{kc}", 0.0) + SEM, "av_va")
            nc.tensor.matmul(
                acc[0:65, o:512],
                va[:, kc, h * 65 : h * 65 + 65],
                expT[:, idx, o:512],
                start=(kc == 0),
                stop=(kc == n - 1),
            )
            pe_busy(mm_cost(w))

        def finish_head(s, h, acc):
            while pending_av:
                emit_av()
            av_done = st["pe"]
            tmp = pool_tmp.tile([128, 512], F32, tag="tmp", name=f"tmp{s}{h}")
            tfin = dve(DVE_TMP, after=av_done)
            nc.vector.tensor_copy(tmp[0:65, :], acc[0:65, :])

            def apply_norm(nfin, bc_ap):
                if h == 0:
                    nc.vector.tensor_mul(
                        outT01[0:64, s * 512 : (s + 1) * 512],
                        tmp[0:64, :],
                        bc_ap,
                    )
                elif h == 2:
                    nc.vector.tensor_mul(
                        outT2[0:64, s * 512 : (s + 1) * 512],
                        tmp[0:64, :],
                        bc_ap,
                    )
                else:
                    stg = pool_stg.tile([64, 512], BF16, tag="stg", name=f"stg{s}")
                    nc.vector.tensor_mul(stg[:], tmp[0:64, :], bc_ap)
                    nc.sync.dma_start(
                        outT01[64:128, s * 512 : (s + 1) * 512], stg[:]
                    )
                    nfin += DMA_LAT
                res_ready[f"outT{s}"] = max(res_ready.get(f"outT{s}", 0.0), nfin)
                if h == 2:
                    queue_proj(s)

            if False and s == NS - 1:
                # tail fast path: PE broadcasts the denominator row from
                # partition 64 down to partitions 0-63 (contraction-1 f32
                # matmul with a ones column), then DVE reciprocal on the
                # broadcast -- skips the partition-hop DMA latency on the
                # critical tail.
                rcp_t = pool_rcp.tile([64, 512], F32, tag="rcp", name=f"rcpt{h}")
                rfin_est = tfin + 900.0

                def norm_unit():
                    need(tfin + SEM, "bcmm")
                    bcp = ps_aux.tile([64, 512], F32, tag="aux", name=f"bcp{h}")
                    nc.tensor.matmul(
                        bcp[:], ones_bc[64:65, 0:64], tmp[64:65, :],
                        start=True, stop=True,
                    )
                    pe_busy(512.0 + MM_OVH + 256.0)  # f32 moving: half rate
                    rfin = dve(DVE_REC, after=st["pe"] + SEM)
                    nc.vector.reciprocal_approx_fast(rcp_t[:], bcp[:])
                    nfin = dve(DVE_NORM, after=rfin)
                    apply_norm(nfin, rcp_t[:])

                return norm_unit, rfin_est

            # steady path: DVE recip and gpsimd partition_broadcast only
            # honor partition 0, so hop the denominator row down via DMA.
            den = pool_den.tile([1, 512], F32, tag="den", name=f"den{s}{h}")
            nc.sync.dma_start(den[:], tmp[64:65, :])
            dfin = tfin + DMA_LAT
            rcp = pool_rcp.tile([1, 512], F32, tag="rcp", name=f"rcp{s}{h}")
            rfin_est = dfin + DVE_REC + 500.0

            def norm_unit():
                rfin = dve(DVE_REC, after=dfin)
                nc.vector.reciprocal_approx_fast(rcp[:], den[:])
                bcast = pool_bc.tile(
                    [64, 512], F32, tag="bcast", name=f"bc{s}{h}"
                )
                bcfin = pool(POOL_BCAST, after=rfin)
                nc.gpsimd.partition_broadcast(bcast[:], rcp[:])
                nfin = dve(DVE_NORM, after=bcfin)
                apply_norm(nfin, bcast[:])

            return norm_unit, rfin_est

        # ================= emission =================
        # measured DMA landing times relative to first warm matmul (~7us
        # real): wqk ~+2.3us, xs strips stream in 10.5-20us real, wp last
        res_ready["wqk"] = 2300.0
        res_ready["wv"] = 6000.0
        res_ready["wp"] = 12800.0
        for s in range(NS):
            res_ready[f"xs{s}"] = 4600.0 + s * 2400.0

        # warmup chain: hold PE activity (HAM) while the first DMAs land.
        # 32-col stationary -> 1/4 array energy, same occupancy.
        warm = ps_aux.tile([32, 512], F32, tag="aux", name="warm")
        NWARM = 12
        for i in range(NWARM):
            nc.tensor.matmul(
                warm[:],
                scratch[:, 0:32],
                scratch[:],
                start=(i == 0),
                stop=(i == NWARM - 1),
            )
            pe_busy(512 * (1.54 if i == 0 else COLD_NS) + MM_OVH)

        # strip-0 prep up front
        add_chain("qk01", qk_chain(0, 1))
        add_chain("qk00", qk_chain(0, 0))
        add_chain("qk02", qk_chain(0, 2))
        for j in range(4):
            add_chain(f"v0{j}", v_chain(0, j))
        for cid in ("qk01", "qk00", "qk02", "v00", "v01", "v02", "v03"):
            force_chain(cid)

        # remaining strips' prep chains: inventory for the pacer, consumed
        # as filler or force-emitted at their use deadlines
        for s2 in range(1, NS):
            add_chain(f"qk{s2}1", qk_chain(s2, 1))
            add_chain(f"qk{s2}0", qk_chain(s2, 0))
            add_chain(f"qk{s2}2", qk_chain(s2, 2))
            for j in range(4):
                add_chain(f"v{s2}{j}", v_chain(s2, j))

        sc_hist = deque(maxlen=3)  # exp-read times of the 3 score banks
        deferred = deque()         # (due_gidx, fn) engine items woven in later

        HEAD_ORDER = [(s, h) for s in range(NS) for h in range(3)]
        for s, h in HEAD_ORDER:
            st["cur_s"] = s
            n = 4 * (s + 1)
            if h == 0:
                force_chain(f"qk{s}1")
                need(res_ready.get(f"qk{s}1", 0.0), "q01")
            if h == 2:
                force_chain(f"qk{s}2")
                need(res_ready.get(f"qk{s}2", 0.0), "qk2")
                need(res_ready.get(f"qt2{s}", 0.0), "qt2")

            acc = ps_acc.tile([128, 512], F32, tag="acc", name=f"acc{s}{h}")
            for kp in range(n // 2):
                pair = (2 * kp, 2 * kp + 1)
                js = [kc - 4 * s for kc in pair]
                os_ = [0 if j < 0 else j * 128 for j in js]
                ws = [512 - o for o in os_]
                for kc, j in zip(pair, js):
                    if j >= 0:
                        force_chain(f"qk{s}0")
                        force_chain(f"v{s}{j}")
                        if h == 2:
                            force_chain(f"qk{s}2")
                        need(
                            res_ready.get(
                                f"qk{s}0" if h < 2 else f"qk{s}2", 0.0
                            ),
                            "kdiag",
                        )
                if len(sc_hist) == 2:
                    need(sc_hist[0] + SEM, "scbank")
                pss = ps_sc.tile([128, 2, 512], F32, tag="sc", name=f"ps{gidx}")
                expT = pool_exp.tile(
                    [128, 2, 512], BF16, tag="expT", name=f"e{gidx}"
                )
                for idx in range(2):
                    lhs, rhs = head_aps(h, pair[idx], s, os_[idx])
                    nc.tensor.matmul(
                        pss[:, idx, os_[idx] : 512], lhs, rhs,
                        start=True, stop=True,
                    )
                    pe_busy(mm_cost(ws[idx]))
                if os_[0] == os_[1]:
                    # one ACT op exps both banks of the pair tile
                    efin = act(exp_cost(2 * ws[0]), after=st["pe"])
                    nc.scalar.activation(
                        expT[:, :, os_[0] : 512], pss[:, :, os_[0] : 512],
                        AF.Exp, scale=SCALE,
                    )
                    efins = [efin, efin]
                else:
                    efins = []
                    for idx in range(2):
                        efins.append(act(exp_cost(ws[idx]), after=st["pe"]))
                        nc.scalar.activation(
                            expT[:, idx, os_[idx] : 512],
                            pss[:, idx, os_[idx] : 512],
                            AF.Exp, scale=SCALE,
                        )
                sc_hist.append(max(efins))  # pair bank frees when exp read it
                for idx in range(2):
                    kc, j, o, w = pair[idx], js[idx], os_[idx], ws[idx]
                    efin = efins[idx]
                    if j >= 0:
                        # DVE, not GpSimd: the GpSimd sequencer burns
                        # 0.3-1.9us per semaphore wait + library reloads
                        efin = dve(DVE_MASK, after=efin)
                        nc.vector.tensor_mul(
                            expT[:, idx, o : o + 128],
                            expT[:, idx, o : o + 128],
                            trimask[:],
                        )
                    exp_fin[gidx] = efin
                    pending_av.append((gidx, h, kc, n, acc, expT, idx, o, w))
                    gidx += 1
                while (
                    deferred
                    and deferred[0][0] <= gidx
                    and deferred[0][1] <= st["pe"]
                ):
                    deferred.popleft()[2]()
                while len(pending_av) > LAG:
                    emit_av()
                while st["act"] > st["pe"] + PACE_SLACK and emit_one_filler():
                    pass

            norm_unit, rfin_est = finish_head(s, h, acc)
            deferred.append((gidx + NORM_LAG, rfin_est, norm_unit))

        # keep the PE clock hot through the tail normalization chain
        # (32-col stationary: 1/4 array energy)
        tail_warm = ps_sc.tile([32, 512], F32, tag="sc", name="tail_warm")
        NTAIL = 16
        for i in range(NTAIL):
            nc.tensor.matmul(
                tail_warm[:],
                scratch[:, 0:32],
                scratch[:],
                start=(i == 0),
                stop=(i == NTAIL - 1),
            )
            pe_busy(mm_cost(512))
        while deferred:
            deferred.popleft()[2]()
        while emit_one_filler():
            pass
        for cid in list(chains):
            force_chain(cid)

        print(
            f"[pacer] pe={st['pe']/1e3:.1f}us (work {st['pe_work']/1e3:.1f}) "
            f"act={st['act']/1e3:.1f}us (busy {st['act_busy']/1e3:.1f}) "
            f"dve busy {st['dve_busy']/1e3:.1f} pool busy {st['pool_busy']/1e3:.1f} "
            f"stall={st['stall']/1e3:.2f}us"
        )
        print("[pacer] stalls:", {k: round(v/1e3, 2) for k, v in sorted(stall_sites.items(), key=lambda kv: -kv[1])})

    nc.compile()
    return nc


def _shard_inputs(x, w_qkv, w_proj):
    bf16 = ml_dtypes.bfloat16
    in_maps = []
    for c in range(8):
        b, g = c // 4, c % 4
        h0 = 3 * g

        def strip_pack(arr2d, cols):
            # [768, cols] -> [128, 6*cols]: row p = concat over eo of
            # arr2d[eo*128 + p, :]
            a = (
                arr2d.reshape(EC, 128, cols)
                .transpose(1, 0, 2)
                .reshape(128, EC * cols)
            )
            return np.ascontiguousarray(a.astype(bf16))

        xT = x[b].T  # [768, 2048]
        m = {}
        for s in range(NS):
            m[f"xs{s}"] = strip_pack(xT[:, s * 512 : (s + 1) * 512], 512)

        q = slice(h0 * D, (h0 + 2) * D)
        k = slice(E + h0 * D, E + (h0 + 2) * D)
        wqk = np.concatenate(
            [
                w_qkv[:, k],                                    # k_h0 | k_h1
                w_qkv[:, q],                                    # q_h0 | q_h1
                w_qkv[:, E + (h0 + 2) * D : E + (h0 + 3) * D],  # k_h2
                w_qkv[:, (h0 + 2) * D : (h0 + 3) * D],          # q_h2
            ],
            axis=1,
        )
        m["wqk"] = strip_pack(wqk, 384)
        wv = w_qkv[:, 2 * E + h0 * D : 2 * E + (h0 + 3) * D]    # v_h0|v_h1|v_h2
        m["wv"] = strip_pack(wv, 192)
        m["wp01"] = np.ascontiguousarray(
            w_proj[h0 * D : (h0 + 2) * D, :].astype(bf16)
        )
        m["wp2"] = np.ascontiguousarray(
            w_proj[(h0 + 2) * D : (h0 + 3) * D, :].astype(bf16)
        )
        in_maps.append(m)
    return in_maps


def kernel(x, w_qkv, w_proj):
    x = np.asarray(x, dtype=np.float32)
    w_qkv = np.asarray(w_qkv, dtype=np.float32)
    w_proj = np.asarray(w_proj, dtype=np.float32)

    if "nc" not in _CACHED:
        _CACHED["nc"] = build_nc()
    nc = _CACHED["nc"]

    in_maps = _shard_inputs(x, w_qkv, w_proj)
    trace = bool(int(os.environ.get("KERNEL_TRACE", "0")))
    res = run_bass_kernel_spmd(nc, in_maps, core_ids=list(range(8)), trace=trace)
    _CACHED["last_results"] = res

    y = np.zeros((2, T, E), dtype=np.float32)
    for c in range(8):
        y[c // 4] += np.asarray(res.results[c]["y"], dtype=np.float32)
    return y

